# revision 1
# baseline (speedup 1.0000x reference)
"""LightGCN (5-layer SpMM propagation + batch lookup) on 8 trn2 NeuronCores.

Strategy (1D row-partition, per sharding hint):
  - Destination rows (nodes) sharded contiguously across 8 cores:
    12500 rows/core, padded to 12544 = 98 tiles of 128.
  - Edges assigned to the core owning their destination row; within a core
    grouped by (dest tile t, source block b) where b indexes 32768-row ranges
    of the padded node space (dma_gather idx is int16).
  - Per layer: each core dma_gathers source rows x[col] (f32, 1KB rows) from
    its full copy of x, multiplies+segment-sums via PE matmul with a
    per-chunk scaled one-hot built on DVE (iota==dloc)*val, accumulates per
    dest tile in PSUM, writes the y shard to DRAM, and AllGathers the shards
    into the next layer's full x.
  - acc = x0/6 + sum_k y_k is never materialized: the final batch rows are
    gathered from x0-shard and the 5 y shards and summed on DVE.

kernel(**inputs) takes the FULL reference inputs and returns
(user_all[users], item_all[items]) exactly like the reference.
"""
import sys, os
sys.path.insert(0, "/opt/trn_rl_repo")
import numpy as np

N_USERS = 50000
N_ITEMS = 50000
N_NODES = N_USERS + N_ITEMS
D = 256
N_LAYERS = 5
N_CORES = 8
ROWS_PER_CORE = N_NODES // N_CORES          # 12500
SHARD = 12544                               # 98 tiles * 128
N_TILES = SHARD // 128                      # 98
N_PAD = SHARD * N_CORES                     # 100352
BLK = 32768
N_BLOCKS = (N_PAD + BLK - 1) // BLK         # 4
GROUP_T = 7                                 # tiles per metadata prefetch group

_CACHE = {}


def _pad_id(g):
    """global node id -> padded node id"""
    return (g // ROWS_PER_CORE) * SHARD + (g % ROWS_PER_CORE)


def _preprocess(adj_vals, adj_row, adj_col, users, items):
    """Build per-core gather/one-hot metadata. Returns dict of host arrays +
    program-structure metadata (identical across cores)."""
    row_p = _pad_id(adj_row.astype(np.int64))
    col_p = _pad_id(adj_col.astype(np.int64))
    core = row_p // SHARD
    loc = row_p % SHARD
    t = loc // 128
    dloc = loc % 128
    b = col_p // BLK
    bidx = col_p % BLK

    # order edges by (core, t, b)
    key = ((core * N_TILES + t) * N_BLOCKS + b)
    order = np.argsort(key, kind="stable")
    key_s = key[order]
    ncells = N_CORES * N_TILES * N_BLOCKS
    counts = np.bincount(key_s, minlength=ncells).reshape(N_CORES, N_TILES, N_BLOCKS)

    # common chunk counts per (t, b): max over cores, >= 1
    C_tb = np.maximum(1, (counts.max(axis=0) + 127) // 128)  # [N_TILES, N_BLOCKS]
    TOTC = int(C_tb.sum())
    seg_chunk_off = np.zeros((N_TILES, N_BLOCKS), dtype=np.int64)
    seg_chunk_off.reshape(-1)[1:] = np.cumsum(C_tb.reshape(-1))[:-1]

    # padded slot position for each edge: seg_base*128 + rank within cell
    cell_start = np.zeros(ncells + 1, dtype=np.int64)
    cell_start[1:] = np.cumsum(counts.reshape(-1))
    rank = np.arange(len(order)) - cell_start[key_s]
    seg_base = np.tile(seg_chunk_off.reshape(-1), N_CORES).reshape(N_CORES, -1)
    slot = seg_base.reshape(-1)[key_s] * 128 + rank   # slot within the core's padded edge space

    idx16 = np.zeros((N_CORES, 128, TOTC * 8), dtype=np.int16)
    dloc_a = np.zeros((N_CORES, 128, TOTC), dtype=np.float32)
    val_a = np.zeros((N_CORES, 128, TOTC), dtype=np.float32)

    bidx_s = bidx[order]
    dloc_s = dloc[order]
    val_s = adj_vals[order].astype(np.float32)
    core_s = core[order]

    # scatter dloc/val: [p, chunk] with p = slot%128, chunk = slot//128
    p_ = (slot % 128).astype(np.int64)
    q_ = (slot // 128).astype(np.int64)
    dloc_a[core_s, p_, q_] = dloc_s.astype(np.float32)
    val_a[core_s, p_, q_] = val_s

    # idx flat per core then wrap per segment
    idx_flat = np.zeros((N_CORES, TOTC * 128), dtype=np.int16)
    idx_flat[core_s, slot] = bidx_s.astype(np.int16)
    flat_off = seg_chunk_off * 128
    for tt in range(N_TILES):
        for bb in range(N_BLOCKS):
            L = int(C_tb[tt, bb]) * 128
            o = int(flat_off[tt, bb])
            seg = idx_flat[:, o : o + L]                      # [8, L]
            w = seg.reshape(N_CORES, L // 16, 16).transpose(0, 2, 1)  # [8,16,L/16]
            idx16[:, :16, o // 16 : (o + L) // 16] = w
    idx16[:, 16:, :] = np.tile(idx16[:, :16, :], (1, 7, 1))

    # ---- final batch gather ----
    nodes = np.concatenate([users.astype(np.int64), items.astype(np.int64) + N_USERS])
    pos = np.arange(len(nodes))
    pn = _pad_id(nodes)
    fcore = pn // SHARD
    floc = pn % SHARD
    n_c = np.bincount(fcore, minlength=N_CORES)
    BPAD = int(128 * ((n_c.max() + 127) // 128))
    fidx = np.zeros((N_CORES, 128, BPAD // 16), dtype=np.int16)
    fpos = []      # per core: batch positions in gather order
    forder = np.argsort(fcore, kind="stable")
    for c in range(N_CORES):
        sel = forder[fcore[forder] == c]
        fl = np.zeros(BPAD, dtype=np.int16)
        fl[: len(sel)] = floc[sel].astype(np.int16)
        fidx[c] = np.tile(fl.reshape(BPAD // 16, 16).T, (8, 1))
        fpos.append(pos[sel])

    # group (metadata prefetch) structure
    groups = []
    for g0 in range(0, N_TILES, GROUP_T):
        tiles = list(range(g0, min(g0 + GROUP_T, N_TILES)))
        c0 = int(seg_chunk_off[tiles[0], 0])
        c1 = int(seg_chunk_off[tiles[-1], -1] + C_tb[tiles[-1], -1])
        groups.append((tiles, c0, c1))
    GC_MAX = max(c1 - c0 for _, c0, c1 in groups)

    meta = dict(
        C_tb=C_tb, seg_chunk_off=seg_chunk_off, TOTC=TOTC, BPAD=BPAD,
        groups=groups, GC_MAX=GC_MAX, CMAX=int(C_tb.max()),
    )
    arrays = dict(idx16=idx16, dloc=dloc_a, val=val_a, fidx=fidx)
    return meta, arrays, fpos


def _build_program(meta):
    from concourse import bass, mybir, tile, library_config
    import concourse.bacc as bacc

    dt = mybir.dt
    C_tb = meta["C_tb"]; seg_off = meta["seg_chunk_off"]
    TOTC = meta["TOTC"]; BPAD = meta["BPAD"]; CMAX = meta["CMAX"]
    groups = meta["groups"]; GC_MAX = meta["GC_MAX"]

    nc = bacc.Bacc("TRN2", target_bir_lowering=False, debug=False,
                   num_devices=N_CORES, num_swdge_queues=4)
    x0 = nc.declare_dram_parameter("x0", [N_PAD, D], dt.float32, isOutput=False)
    x0s = nc.declare_dram_parameter("x0s", [SHARD, D], dt.float32, isOutput=False)
    idxp = nc.declare_dram_parameter("idx", [128, TOTC * 8], dt.int16, isOutput=False)
    dlocp = nc.declare_dram_parameter("dloc", [128, TOTC], dt.float32, isOutput=False)
    valp = nc.declare_dram_parameter("val", [128, TOTC], dt.float32, isOutput=False)
    fidxp = nc.declare_dram_parameter("fidx", [128, BPAD // 16], dt.int16, isOutput=False)
    iotap = nc.declare_dram_parameter("iota", [128, CMAX * 128], dt.float32, isOutput=False)
    outp = nc.declare_dram_parameter("out", [128, BPAD // 128, D], dt.float32, isOutput=True)

    blocks = [(bb * BLK, min((bb + 1) * BLK, N_PAD)) for bb in range(N_BLOCKS)]

    with tile.TileContext(nc) as tc:
        nc.gpsimd.load_library(library_config.mlp)
        with (
            tc.tile_pool(name="dram", bufs=1, space="DRAM") as dpool,
            tc.tile_pool(name="const", bufs=1) as cpool,
            tc.tile_pool(name="gb", bufs=3) as gpool,
            tc.tile_pool(name="oh", bufs=3) as opool,
            tc.tile_pool(name="meta", bufs=2) as mpool,
            tc.tile_pool(name="y", bufs=3) as ypool,
            tc.tile_pool(name="fin", bufs=2) as fpool,
            tc.tile_pool(name="ps", bufs=4, space="PSUM") as ppool,
        ):
            ydram = [dpool.tile([SHARD, D], dt.float32, tag=f"y{k}",
                                name=f"ydram{k}")
                     for k in range(N_LAYERS)]
            agbuf = [dpool.tile([N_PAD, D], dt.float32, tag=f"ag{i}",
                                name=f"agbuf{i}")
                     for i in range(2)]

            iota_f = cpool.tile([128, CMAX, 128], dt.float32)
            nc.sync.dma_start(iota_f[:], iotap[:].rearrange("p (c j) -> p c j", j=128))

            for k in range(N_LAYERS):
                x_src = x0 if k == 0 else agbuf[(k - 1) % 2]
                for tiles, c0, c1 in groups:
                    it = mpool.tile([128, GC_MAX * 8], dt.int16, tag="it")
                    nc.sync.dma_start(it[:, : (c1 - c0) * 8],
                                      idxp[:, c0 * 8 : c1 * 8])
                    dl = mpool.tile([128, GC_MAX], dt.float32, tag="dl")
                    nc.sync.dma_start(dl[:, : c1 - c0], dlocp[:, c0:c1])
                    vl = mpool.tile([128, GC_MAX], dt.float32, tag="vl")
                    nc.sync.dma_start(vl[:, : c1 - c0], valp[:, c0:c1])

                    for t in tiles:
                        psum = ppool.tile([128, D], dt.float32)
                        for bb in range(N_BLOCKS):
                            C = int(C_tb[t, bb])
                            co = int(seg_off[t, bb])          # global chunk off
                            lo = co - c0                       # within group slice
                            gb = gpool.tile([128, CMAX, D], dt.float32, tag="gb")
                            nc.gpsimd.dma_gather(
                                gb[:, :C, :],
                                x_src[blocks[bb][0] : blocks[bb][1]],
                                it[:, co * 8 - c0 * 8 : (co + C) * 8 - c0 * 8],
                                C * 128, C * 128, D, single_packet=False,
                                queue_num=(t * N_BLOCKS + bb) % 4)
                            oh = opool.tile([128, CMAX, 128], dt.float32, tag="oh")
                            nc.vector.tensor_tensor(
                                out=oh[:, :C, :], in0=iota_f[:, :C, :],
                                in1=dl[:, lo : lo + C].to_broadcast([128, C, 128]),
                                op=mybir.AluOpType.is_equal)
                            nc.vector.tensor_tensor(
                                out=oh[:, :C, :], in0=oh[:, :C, :],
                                in1=vl[:, lo : lo + C].to_broadcast([128, C, 128]),
                                op=mybir.AluOpType.mult)
                            for q in range(C):
                                nc.tensor.matmul(
                                    psum[:], oh[:, q, :], gb[:, q, :],
                                    start=(bb == 0 and q == 0),
                                    stop=(bb == N_BLOCKS - 1 and q == C - 1))
                        ysb = ypool.tile([128, D], dt.float32, tag="ysb")
                        nc.scalar.copy(ysb[:], psum[:])
                        nc.sync.dma_start(ydram[k][t * 128 : (t + 1) * 128, :], ysb[:])
                nc.gpsimd.collective_compute(
                    "AllGather", mybir.AluOpType.bypass,
                    ins=[ydram[k].opt()],
                    outs=[agbuf[k % 2].opt()],
                    replica_groups=[list(range(N_CORES))])

            # ---- final batch gather + sum over (x0 shard, y_1..y_5) ----
            fit = fpool.tile([128, BPAD // 16], dt.int16, tag="fit")
            nc.sync.dma_start(fit[:], fidxp[:])
            facc = fpool.tile([128, BPAD // 128, D], dt.float32, tag="facc")
            nc.gpsimd.dma_gather(facc[:], x0s[:], fit[:], BPAD, BPAD, D,
                                 single_packet=False)
            for k in range(N_LAYERS):
                ftmp = fpool.tile([128, BPAD // 128, D], dt.float32, tag="ftmp")
                nc.gpsimd.dma_gather(ftmp[:], ydram[k][:], fit[:], BPAD, BPAD, D,
                                     single_packet=False)
                nc.vector.tensor_add(out=facc[:], in0=facc[:], in1=ftmp[:])
            nc.sync.dma_start(outp[:], facc[:])

    nc.compile()
    return nc


def kernel(user_table, item_table, adj_vals, adj_row, adj_col, users, items,
           trace=False):
    from concourse.bass_utils import run_bass_kernel_spmd

    user_table = np.asarray(user_table, dtype=np.float32)
    item_table = np.asarray(item_table, dtype=np.float32)
    adj_vals = np.asarray(adj_vals, dtype=np.float32)
    adj_row = np.asarray(adj_row).astype(np.int64)
    adj_col = np.asarray(adj_col).astype(np.int64)
    users_i = np.asarray(users).astype(np.int64)
    items_i = np.asarray(items).astype(np.int64)

    meta, arrays, fpos = _preprocess(adj_vals, adj_row, adj_col, users_i, items_i)

    ck = (meta["TOTC"], meta["BPAD"], meta["C_tb"].tobytes())
    if ck not in _CACHE:
        _CACHE[ck] = _build_program(meta)
    nc = _CACHE[ck]

    x0 = np.concatenate([user_table, item_table], axis=0) / 6.0
    x0_pad = np.zeros((N_PAD, D), dtype=np.float32)
    pads = _pad_id(np.arange(N_NODES))
    x0_pad[pads] = x0

    in_maps = []
    for c in range(N_CORES):
        in_maps.append({
            "x0": x0_pad,
            "x0s": x0_pad[c * SHARD : (c + 1) * SHARD],
            "idx": arrays["idx16"][c],
            "dloc": arrays["dloc"][c],
            "val": arrays["val"][c],
            "fidx": arrays["fidx"][c],
            "iota": np.tile(np.arange(128, dtype=np.float32),
                            (128, meta["CMAX"])),
        })

    res = run_bass_kernel_spmd(nc, in_maps, core_ids=list(range(N_CORES)),
                               trace=trace)

    out_full = np.zeros((2 * len(users_i), D), dtype=np.float32)
    for c in range(N_CORES):
        ob = res.results[c]["out"]          # [128, BPAD//128, D]
        p = fpos[c]
        j = np.arange(len(p))
        out_full[p] = ob[j % 128, j // 128, :]
    B = len(users_i)
    ret = (out_full[:B], out_full[B:])
    if trace:
        return ret, res
    return ret

